# revision 10
# baseline (speedup 1.0000x reference)
"""Trainium2 Bass kernel for a 2-layer GRU char autoencoder (B=512, S=512, V=99, E=H=256).

Sharding: pure data-parallel over batch, 8 cores x 64 rows each.

Per-core design (all fp32):
  - Hidden states kept stacked on partitions ([128, 256]: rows 0:64 = layer0,
    64:128 = layer1) plus transposed [128, 128] tiles (two 128-row chunks side
    by side) as the stationary matmul operand for h @ Whh.T.
  - The layer-0 input matmul is fused with the embedding lookup:
    gi0 = onehot_aug @ [emb @ Wih0.T ; biases], contraction over V+1 = 100.
    Encoder one-hots are host-precomputed; decoder one-hots come from
    vector.max + is_equal + a PE transpose, entirely on-chip.
  - Encoder runs its two layers software-pipelined (layer 1 lags one step) with
    column-tiled concurrent matmuls (emission interleaved across the two
    column-group lanes so their streams overlap), so both layers' gates are
    computed by single [128, *] vector/scalar ops.
  - Decoder is serial per step: d0 cell -> d1 cell -> fc -> argmax -> one-hot.
    Gate math is split into 128-column halves to pipeline the serial chain,
    and the next step's recurrent matmuls are pre-emitted to fill PE gaps.
"""

import sys
import numpy as np

if "/opt/trn_rl_repo" not in sys.path:
    sys.path.insert(0, "/opt/trn_rl_repo")

V, E, H = 99, 256, 256
B, S = 512, 512
NCORES = 8
BL = B // NCORES  # 64 rows per core

_PROGRAM_CACHE = {}


def _build_program(repeat=1, staggered=True):
    import contextlib
    import concourse.bass as bass
    import concourse.bacc as bacc
    import concourse.mybir as mybir
    from concourse.tile import TileContext

    f32 = mybir.dt.float32
    AF = mybir.ActivationFunctionType
    ALU = mybir.AluOpType

    nc = bacc.Bacc("TRN2", target_bir_lowering=False, debug=False,
                   num_devices=NCORES)

    # ---- DRAM I/O ----
    din = {}
    for name, shape in [
        ("oh_enc", [S * 100, BL]),     # per-core: encoder one-hot^T (aug) per step
        ("oh_dec0", [100, BL]),        # per-core: initial decoder one-hot^T
        ("iden", [128, 64]),           # two stacked 64x64 identities
        ("wf_e0", [100, 1024]),        # fused emb@Wih0^T + biases (enc layer0)
        ("wf_d0", [100, 1024]),        # same for dec layer0
        ("whhT_e0", [256, 768]),
        ("whhT_e1", [256, 768]),
        ("whhT_d0", [256, 768]),
        ("whhT_d1", [256, 768]),
        ("wihT_e1", [256, 768]),
        ("wihT_d1", [256, 768]),
        ("bias_e1", [1, 1024]),
        ("bias_d1", [1, 1024]),
        ("fcwT", [256, V]),
        ("fcb", [BL, V]),
    ]:
        din[name] = nc.dram_tensor(name, shape, f32, kind="ExternalInput")
    dout = nc.dram_tensor("out", [BL, S * V], f32, kind="ExternalOutput")

    with TileContext(nc) as tc:
        # ---- persistent SBUF state ----
        def sb(name, shape):
            return nc.alloc_sbuf_tensor(name, shape, f32).ap()

        hA = sb("hA", [128, 256])       # states stacked: rows 0:64 = l0, 64:128 = l1
        hB = sb("hB", [128, 256])
        h0T = sb("h0T", [128, 128])     # transposed l0 state (c0 | c1)
        h1T = sb("h1T", [128, 128])
        ohdec = sb("ohdec", [100, BL])  # decoder one-hot^T (row 99 stays 1.0)
        ones = sb("ones1", [1, BL])
        iden = sb("iden_sb", [128, 64])

        nc.vector.memset(ones[:], 1.0)
        nc.sync.dma_start(iden[:], din["iden"][:])

        with tc.tile_pool(name="wp", bufs=1) as wp:
            # ---- load weights into SBUF once ----
            def wtile(name, shape, src):
                t = wp.tile(shape, f32, tag=name)
                nc.sync.dma_start(t[:], src)
                return t

            wf_e0 = wtile("wf_e0", [100, 1024], din["wf_e0"][:])
            wf_d0 = wtile("wf_d0", [100, 1024], din["wf_d0"][:])
            whh = {}
            for l in ("e0", "e1", "d0", "d1"):
                for c in (0, 1):
                    whh[l, c] = wtile(f"whh_{l}_{c}", [128, 768],
                                      din[f"whhT_{l}"][c * 128:(c + 1) * 128, :])
            wih = {}
            for l in ("e1", "d1"):
                for c in (0, 1):
                    wih[l, c] = wtile(f"wih_{l}_{c}", [128, 768],
                                      din[f"wihT_{l}"][c * 128:(c + 1) * 128, :])
            bias_e1 = wtile("bias_e1", [1, 1024], din["bias_e1"][:])
            bias_d1 = wtile("bias_d1", [1, 1024], din["bias_d1"][:])
            fcw = {c: wtile(f"fcw_{c}", [128, V],
                            din["fcwT"][c * 128:(c + 1) * 128, :]) for c in (0, 1)}
            fcb = wtile("fcb", [BL, V], din["fcb"][:])

            MM = nc.tensor.matmul

            def repeat_loop():
                if repeat == 1:
                    return contextlib.nullcontext(0)
                return tc.For_i(0, repeat, 1)

            # ---- matmul emission helpers: return lists of closures so the two
            # column-group lanes can be interleaved (their streams overlap) ----
            def cell_list(prz, pghn, whh_l, hT, col):
                r0, r1 = col, col + 64
                tp = (0, col)
                return [
                    lambda: MM(prz[r0:r1, :], lhsT=hT[:, 0:64],
                               rhs=whh_l[0][:, 0:512], start=True, stop=False,
                               tile_position=tp),
                    lambda: MM(prz[r0:r1, :], lhsT=hT[:, 64:128],
                               rhs=whh_l[1][:, 0:512], start=False, stop=False,
                               tile_position=tp),
                    lambda: MM(pghn[r0:r1, :], lhsT=hT[:, 0:64],
                               rhs=whh_l[0][:, 512:768], start=True, stop=False,
                               tile_position=tp),
                    lambda: MM(pghn[r0:r1, :], lhsT=hT[:, 64:128],
                               rhs=whh_l[1][:, 512:768], start=False, stop=False,
                               tile_position=tp),
                ]

            def fused_list(prz, pgin, pghn, wf, oh, col):
                r0, r1 = col, col + 64
                tp = (0, col)
                return [
                    lambda: MM(prz[r0:r1, :], lhsT=oh[:], rhs=wf[:, 0:512],
                               start=False, stop=True, tile_position=tp),
                    lambda: MM(pgin[r0:r1, :], lhsT=oh[:], rhs=wf[:, 512:768],
                               start=True, stop=True, tile_position=tp),
                    lambda: MM(pghn[r0:r1, :], lhsT=oh[:], rhs=wf[:, 768:1024],
                               start=False, stop=True, tile_position=tp),
                ]

            def l1_input_list(prz, pgin, wih_l, xT, col, rz_stop, gin_start,
                              gin_stop):
                r0, r1 = col, col + 64
                tp = (0, col)
                return [
                    lambda: MM(prz[r0:r1, :], lhsT=xT[:, 0:64],
                               rhs=wih_l[0][:, 0:512], start=False, stop=False,
                               tile_position=tp),
                    lambda: MM(prz[r0:r1, :], lhsT=xT[:, 64:128],
                               rhs=wih_l[1][:, 0:512], start=False, stop=rz_stop,
                               tile_position=tp),
                    lambda: MM(pgin[r0:r1, :], lhsT=xT[:, 0:64],
                               rhs=wih_l[0][:, 512:768], start=gin_start,
                               stop=False, tile_position=tp),
                    lambda: MM(pgin[r0:r1, :], lhsT=xT[:, 64:128],
                               rhs=wih_l[1][:, 512:768], start=False,
                               stop=gin_stop, tile_position=tp),
                ]

            def l1_bias_list(prz, pgin, pghn, bias_l, col, rz_stop, gin_start,
                             gin_stop):
                r0, r1 = col, col + 64
                tp = (0, col)
                return [
                    lambda: MM(prz[r0:r1, :], lhsT=ones[:], rhs=bias_l[:, 0:512],
                               start=False, stop=rz_stop, tile_position=tp),
                    lambda: MM(pgin[r0:r1, :], lhsT=ones[:],
                               rhs=bias_l[:, 512:768], start=gin_start,
                               stop=gin_stop, tile_position=tp),
                    lambda: MM(pghn[r0:r1, :], lhsT=ones[:],
                               rhs=bias_l[:, 768:1024], start=False, stop=True,
                               tile_position=tp),
                ]

            def emit_lanes(lane_a, lane_b):
                """Interleave two column-group lanes; lane_a gets priority."""
                n = max(len(lane_a), len(lane_b))
                for i in range(n):
                    if i < len(lane_a):
                        lane_a[i]()
                    if i < len(lane_b):
                        lane_b[i]()

            def gates(rows, prz, pgin, pghn, src, dst, gp):
                """GRU gate math + state update (unsplit, for the encoder)."""
                r0, r1 = rows
                rz = gp.tile([128, 512], f32, tag="rz")
                nc.scalar.activation(rz[r0:r1, :], prz[r0:r1, :], AF.Sigmoid)
                m1 = gp.tile([128, 256], f32, tag="m1")
                nc.vector.tensor_tensor(m1[r0:r1, :], in0=rz[r0:r1, 0:256],
                                        in1=pghn[r0:r1, :], op=ALU.mult)
                npre = gp.tile([128, 256], f32, tag="npre")
                nc.vector.tensor_tensor(npre[r0:r1, :], in0=m1[r0:r1, :],
                                        in1=pgin[r0:r1, :], op=ALU.add)
                nt = gp.tile([128, 256], f32, tag="nt")
                nc.scalar.activation(nt[r0:r1, :], npre[r0:r1, :], AF.Tanh)
                t1 = gp.tile([128, 256], f32, tag="t1")
                nc.vector.scalar_tensor_tensor(t1[r0:r1, :], in0=rz[r0:r1, 256:512],
                                               scalar=1.0, in1=nt[r0:r1, :],
                                               op0=ALU.subtract, op1=ALU.mult)
                t2 = gp.tile([128, 256], f32, tag="t2")
                nc.vector.tensor_tensor(t2[r0:r1, :], in0=rz[r0:r1, 256:512],
                                        in1=src[r0:r1, :], op=ALU.mult)
                nc.vector.tensor_tensor(dst[r0:r1, :], in0=t2[r0:r1, :],
                                        in1=t1[r0:r1, :], op=ALU.subtract)

            def transpose_state(hrow, base, pta, ptb, dest):
                """PE-transpose a [64, 256] state block (at partition base) into
                dest [128, 128] = (dims 0:128 | dims 128:256)^T. One PSUM bank
                per 128x64 transpose (start=True clears the whole bank line)."""
                idn = iden[base:base + 64, :]
                nc.tensor.transpose(pta[:], hrow[:, 0:128], idn)
                nc.tensor.transpose(ptb[:], hrow[:, 128:256], idn)
                nc.vector.tensor_copy(dest[:, 0:64], pta[:])
                nc.vector.tensor_copy(dest[:, 64:128], ptb[:])

            def gates_dec(rows, prz, pgin, pghn, src, dst, gp, pta, ptb, destT,
                          base):
                """Decoder cell: gate math split into 128-column halves so the
                serial chain pipelines across ACT/DVE/PE, with each transposed
                state chunk emitted as soon as its half is ready."""
                r0, r1 = rows
                rz = gp.tile([128, 512], f32, tag="rz")
                # r first (m1 needs it), z second (needed later by t1/t2)
                nc.scalar.activation(rz[r0:r1, 0:256], prz[r0:r1, 0:256],
                                     AF.Sigmoid)
                nc.scalar.activation(rz[r0:r1, 256:512], prz[r0:r1, 256:512],
                                     AF.Sigmoid)
                m1 = gp.tile([128, 256], f32, tag="m1")
                npre = gp.tile([128, 256], f32, tag="npre")
                nt = gp.tile([128, 256], f32, tag="nt")
                t1 = gp.tile([128, 256], f32, tag="t1")
                t2 = gp.tile([128, 256], f32, tag="t2")
                idn = iden[base:base + 64, :]
                ca, cb = slice(0, 128), slice(128, 256)
                za, zb = slice(256, 384), slice(384, 512)
                # DVE work interleaved across the halves so half-b's n-path
                # streams while ACT runs half-a's tanh
                nc.vector.tensor_tensor(m1[r0:r1, ca], in0=rz[r0:r1, ca],
                                        in1=pghn[r0:r1, ca], op=ALU.mult)
                nc.vector.tensor_tensor(npre[r0:r1, ca], in0=m1[r0:r1, ca],
                                        in1=pgin[r0:r1, ca], op=ALU.add)
                nc.scalar.activation(nt[r0:r1, ca], npre[r0:r1, ca], AF.Tanh)
                nc.vector.tensor_tensor(m1[r0:r1, cb], in0=rz[r0:r1, cb],
                                        in1=pghn[r0:r1, cb], op=ALU.mult)
                nc.vector.tensor_tensor(npre[r0:r1, cb], in0=m1[r0:r1, cb],
                                        in1=pgin[r0:r1, cb], op=ALU.add)
                nc.scalar.activation(nt[r0:r1, cb], npre[r0:r1, cb], AF.Tanh)
                nc.vector.tensor_tensor(t2[r0:r1, ca], in0=rz[r0:r1, za],
                                        in1=src[r0:r1, ca], op=ALU.mult)
                nc.vector.tensor_tensor(t2[r0:r1, cb], in0=rz[r0:r1, zb],
                                        in1=src[r0:r1, cb], op=ALU.mult)
                nc.vector.scalar_tensor_tensor(t1[r0:r1, ca], in0=rz[r0:r1, za],
                                               scalar=1.0, in1=nt[r0:r1, ca],
                                               op0=ALU.subtract, op1=ALU.mult)
                nc.vector.tensor_tensor(dst[r0:r1, ca], in0=t2[r0:r1, ca],
                                        in1=t1[r0:r1, ca], op=ALU.subtract)
                nc.tensor.transpose(pta[:], dst[r0:r1, ca], idn)
                nc.vector.scalar_tensor_tensor(t1[r0:r1, cb], in0=rz[r0:r1, zb],
                                               scalar=1.0, in1=nt[r0:r1, cb],
                                               op0=ALU.subtract, op1=ALU.mult)
                nc.vector.tensor_tensor(dst[r0:r1, cb], in0=t2[r0:r1, cb],
                                        in1=t1[r0:r1, cb], op=ALU.subtract)
                nc.tensor.transpose(ptb[:], dst[r0:r1, cb], idn)
                nc.vector.tensor_copy(destT[:, 0:64], pta[:])
                nc.vector.tensor_copy(destT[:, 64:128], ptb[:])

            with repeat_loop():
              # state init (inside the repeat loop so reps are identical)
              nc.vector.memset(hA[:], 0.0)
              nc.vector.memset(hB[:], 0.0)
              nc.vector.memset(h0T[:], 0.0)
              nc.vector.memset(h1T[:], 0.0)
              nc.sync.dma_start(ohdec[:], din["oh_dec0"][:])
              # ================= ENCODER =================
              with (
                  tc.tile_pool(name="eoh", bufs=3) as eoh,
                  tc.tile_pool(name="eg", bufs=2) as eg,
                  tc.tile_pool(name="eps2", bufs=2, space="PSUM") as eps2,
                  tc.tile_pool(name="eps1", bufs=1, space="PSUM") as eps1,
                  tc.tile_pool(name="ept", bufs=1, space="PSUM") as ept,
              ):
                  def enc_round(r, oh_off, l0, l1):
                      """One pipelined encoder round (layer0 = step r, layer1 =
                      step r-1). r is only used for state ping-pong parity."""
                      src = (hA, hB)[r % 2]
                      dst = (hA, hB)[(r + 1) % 2]
                      if l0:
                          ohe = eoh.tile([100, BL], f32, tag="ohe")
                          nc.sync.dma_start(
                              ohe[:], din["oh_enc"][bass.ds(oh_off, 100), :])
                      prz = eps2.tile([128, 512], f32, tag="prz", space="PSUM")
                      pgin = eps1.tile([128, 256], f32, tag="pgin", space="PSUM")
                      pghn = eps1.tile([128, 256], f32, tag="pghn", space="PSUM")
                      lane0, lane1 = [], []
                      if l0:
                          lane0 = (cell_list(prz, pghn,
                                             (whh["e0", 0], whh["e0", 1]), h0T, 0)
                                   + fused_list(prz, pgin, pghn, wf_e0, ohe, 0))
                      if l1:
                          lane1 = (cell_list(prz, pghn,
                                             (whh["e1", 0], whh["e1", 1]), h1T, 64)
                                   + l1_input_list(prz, pgin,
                                                   (wih["e1", 0], wih["e1", 1]),
                                                   h0T, 64, rz_stop=False,
                                                   gin_start=True, gin_stop=False)
                                   + l1_bias_list(prz, pgin, pghn, bias_e1, 64,
                                                  rz_stop=True, gin_start=False,
                                                  gin_stop=True))
                      emit_lanes(lane1, lane0)
                      rows = ((0, 128) if (l0 and l1)
                              else ((0, 64) if l0 else (64, 128)))
                      gates(rows, prz, pgin, pghn, src, dst, eg)
                      if l0:
                          pta = ept.tile([128, 64], f32, tag="pt1a", space="PSUM")
                          ptb = ept.tile([128, 64], f32, tag="pt1b", space="PSUM")
                          transpose_state(dst[0:64, :], 0, pta, ptb, h0T)
                      if l1:
                          ptc = ept.tile([128, 64], f32, tag="pt2a", space="PSUM")
                          ptd = ept.tile([128, 64], f32, tag="pt2b", space="PSUM")
                          transpose_state(dst[64:128, :], 64, ptc, ptd, h1T)

                  # round 0: layer 0 only
                  enc_round(0, 0, True, False)
                  # rounds 1..510 in a hardware loop, 10 per iteration
                  with tc.For_i(100, 51100, 1000,
                                hint_engines=(mybir.EngineType.PE,),
                                staggered_reset=staggered) as iv:
                      for u in range(10):
                          enc_round(1 + u, iv + u * 100, True, True)
                  # round 511 (both layers), round 512 (layer 1 only)
                  enc_round(511, 51100, True, True)
                  enc_round(512, None, False, True)

              # decoder initial state: d0 = e0 (already hA rows 0:64, round 511),
              # d1 = e1 (hB rows 64:128, round 512) -> copy into hA rows 64:128.
              nc.vector.tensor_copy(hA[64:128, :], hB[64:128, :])
              d0T, d1T = h0T, h1T  # hold e0^T / e1^T already

              # ================= DECODER =================
              with (
                  tc.tile_pool(name="dg", bufs=2) as dg,
                  tc.tile_pool(name="dps2", bufs=2, space="PSUM") as dps2,
                  tc.tile_pool(name="dps1", bufs=1, space="PSUM") as dps1,
                  tc.tile_pool(name="dpt", bufs=1, space="PSUM") as dpt,
              ):
                  def step_tiles():
                      prz = dps2.tile([128, 512], f32, tag="prz", space="PSUM")
                      pgin = dps1.tile([128, 256], f32, tag="pgin", space="PSUM")
                      pghn = dps1.tile([128, 256], f32, tag="pghn", space="PSUM")
                      return (prz, pgin, pghn)

                  def dec_step(s, out_off, tiles, gh0_done, nxt):
                      src = (hA, hB)[s % 2]
                      dst = (hA, hB)[(s + 1) % 2]
                      prz, pgin, pghn = tiles
                      # d0 lane: gh (unless pre-emitted last step) + fused input;
                      # d1 lane: gh + biases (ready early; streams overlap d0's)
                      lane0 = [] if gh0_done else cell_list(
                          prz, pghn, (whh["d0", 0], whh["d0", 1]), d0T, 0)
                      lane0 = lane0 + fused_list(prz, pgin, pghn, wf_d0, ohdec, 0)
                      lane1 = (cell_list(prz, pghn, (whh["d1", 0], whh["d1", 1]),
                                         d1T, 64)
                               + l1_bias_list(prz, pgin, pghn, bias_d1, 64,
                                              rz_stop=False, gin_start=True,
                                              gin_stop=False))
                      emit_lanes(lane0, lane1)
                      # d0 gates (+ d0T transposes inside)
                      pta = dpt.tile([128, 64], f32, tag="ptda", space="PSUM")
                      ptb = dpt.tile([128, 64], f32, tag="ptdb", space="PSUM")
                      gates_dec((0, 64), prz, pgin, pghn, src, dst, dg,
                                pta, ptb, d0T, 0)
                      # d1 input side from d0'
                      for fn in l1_input_list(prz, pgin,
                                              (wih["d1", 0], wih["d1", 1]), d0T,
                                              64, rz_stop=True, gin_start=False,
                                              gin_stop=True):
                          fn()
                      # d1 gates (+ d1T transposes inside)
                      ptc = dpt.tile([128, 64], f32, tag="ptda", space="PSUM")
                      ptd = dpt.tile([128, 64], f32, tag="ptdb", space="PSUM")
                      gates_dec((64, 128), prz, pgin, pghn, src, dst, dg,
                                ptc, ptd, d1T, 64)
                      # fc logits
                      plog = dpt.tile([64, V], f32, tag="plog", space="PSUM")
                      MM(plog[:], lhsT=d1T[:, 0:64], rhs=fcw[0][:],
                         start=True, stop=False)
                      MM(plog[:], lhsT=d1T[:, 64:128], rhs=fcw[1][:],
                         start=False, stop=True)
                      # pre-emit next step's d0 recurrent matmuls into the PE
                      # gap while the argmax path runs on ACT/DVE
                      if nxt is not None:
                          for fn in cell_list(nxt[0], nxt[2],
                                              (whh["d0", 0], whh["d0", 1]),
                                              d0T, 0):
                              fn()
                      lg = dg.tile([64, V], f32, tag="lg")
                      nc.vector.tensor_tensor(lg[:], in0=plog[:], in1=fcb[:],
                                              op=ALU.add)
                      nc.sync.dma_start(dout[:, bass.ds(out_off, V)], lg[:])
                      # argmax -> one-hot -> transpose into ohdec
                      m8 = dg.tile([64, 8], f32, tag="m8")
                      nc.vector.max(m8[:], lg[:])
                      oht = dg.tile([64, V], f32, tag="oht")
                      nc.vector.tensor_scalar(oht[:], lg[:], m8[:, 0:1], None,
                                              ALU.is_equal)
                      ptoh = dpt.tile([V, 64], f32, tag="ptoh", space="PSUM")
                      nc.tensor.transpose(ptoh[:], oht[:], iden[0:64, :])
                      nc.vector.tensor_copy(ohdec[0:V, :], ptoh[:])

                  with tc.For_i(0, S * V, 8 * V,
                                hint_engines=(mybir.EngineType.PE,),
                                staggered_reset=staggered) as iv:
                      tiles = step_tiles()
                      gh0_done = False
                      for u in range(8):
                          nxt = step_tiles() if u < 7 else None
                          dec_step(u, iv + u * V, tiles, gh0_done, nxt)
                          gh0_done = nxt is not None
                          if nxt is not None:
                              tiles = nxt

    nc.compile()
    return nc


def _host_prep(inputs):
    f32 = np.float32
    seq = np.asarray(inputs["input_seq"]).astype(np.int64)
    emb = np.asarray(inputs["embedding"], dtype=f32)

    def fused_l0(Wih, bih, bhh):
        M = emb @ np.asarray(Wih, f32).T  # [99, 768]
        wf = np.zeros((100, 1024), f32)
        wf[:V, 0:768] = M
        bih = np.asarray(bih, f32)
        bhh = np.asarray(bhh, f32)
        wf[V, 0:512] = bih[0:512] + bhh[0:512]
        wf[V, 512:768] = bih[512:768]
        wf[V, 768:1024] = bhh[512:768]
        return wf

    def bias_l1(bih, bhh):
        bih = np.asarray(bih, f32)
        bhh = np.asarray(bhh, f32)
        b = np.zeros((1, 1024), f32)
        b[0, 0:512] = bih[0:512] + bhh[0:512]
        b[0, 512:768] = bih[512:768]
        b[0, 768:1024] = bhh[512:768]
        return b

    shared = {
        "iden": np.concatenate([np.eye(64, dtype=f32), np.eye(64, dtype=f32)], 0),
        "wf_e0": fused_l0(inputs["enc_Wih0"], inputs["enc_bih0"], inputs["enc_bhh0"]),
        "wf_d0": fused_l0(inputs["dec_Wih0"], inputs["dec_bih0"], inputs["dec_bhh0"]),
        "whhT_e0": np.ascontiguousarray(np.asarray(inputs["enc_Whh0"], f32).T),
        "whhT_e1": np.ascontiguousarray(np.asarray(inputs["enc_Whh1"], f32).T),
        "whhT_d0": np.ascontiguousarray(np.asarray(inputs["dec_Whh0"], f32).T),
        "whhT_d1": np.ascontiguousarray(np.asarray(inputs["dec_Whh1"], f32).T),
        "wihT_e1": np.ascontiguousarray(np.asarray(inputs["enc_Wih1"], f32).T),
        "wihT_d1": np.ascontiguousarray(np.asarray(inputs["dec_Wih1"], f32).T),
        "bias_e1": bias_l1(inputs["enc_bih1"], inputs["enc_bhh1"]),
        "bias_d1": bias_l1(inputs["dec_bih1"], inputs["dec_bhh1"]),
        "fcwT": np.ascontiguousarray(np.asarray(inputs["fc_W"], f32).T),
        "fcb": np.ascontiguousarray(
            np.broadcast_to(np.asarray(inputs["fc_b"], f32), (BL, V))),
    }

    in_maps = []
    ar_s = np.arange(S)[:, None]
    ar_b = np.arange(BL)[None, :]
    for c in range(NCORES):
        rows = seq[c * BL:(c + 1) * BL]  # [64, 512]
        ohe = np.zeros((S, 100, BL), f32)
        ohe[ar_s, rows.T, ar_b] = 1.0
        ohe[:, 99, :] = 1.0
        ohd = np.zeros((100, BL), f32)
        ohd[rows[:, 0], np.arange(BL)] = 1.0
        ohd[99, :] = 1.0
        m = dict(shared)
        m["oh_enc"] = ohe.reshape(S * 100, BL)
        m["oh_dec0"] = ohd
        in_maps.append(m)
    return in_maps


def kernel(**inputs):
    from concourse.bass_utils import run_bass_kernel_spmd

    if "nc" not in _PROGRAM_CACHE:
        _PROGRAM_CACHE["nc"] = _build_program()
    nc = _PROGRAM_CACHE["nc"]

    in_maps = _host_prep(inputs)
    res = run_bass_kernel_spmd(nc, in_maps, core_ids=list(range(NCORES)))
    out = np.concatenate(
        [res.results[c]["out"].reshape(BL, S, V) for c in range(NCORES)], axis=0)
    return out
